# revision 26
# baseline (speedup 1.0000x reference)
"""Trainium2 Bass kernel: normalized min-sum LDPC decoder (nn_Decoding_model).

Sharding: pure batch data-parallelism. B=16 rows split across 8 NeuronCores
(2 rows/core); H-derived matrices are replicated per core.

Per core, per iteration (BL=2 batch rows):
  A_bcast[b] : [128, N] f32, each partition holds |so_b| (PE transpose of the
               column-major state + DRAM-broadcast DMA)
  neg_v      : per m-chunk [128, N]: hbig(fp16, NEG_BACK on non-edges) - A_bcast
  vmax8      : DVE top-8 of neg_v per check row -> m1=-v0, m2=-v1 (dup-exact)
  onehot     : (neg_v == v0) bf16  (argmin edges; ties give t2=0, harmless)
  parity     : P[m] = sum_n H^T[n,m]*(so[n]<0)  (fp8 matmul, exact counts)
  rs = 1-2*(P mod 2);  t1 = rs*m1;  t2 = rs*(m2-m1)  (3-way bf16 splits)
  main[n]    : sum_m H[m,n]*t1[m]       (bf16 H stationary, fp32 PSUM)
  corr[n]    : sum_m onehot[m,n]*t2[m]  (bf16 onehot stationary)
  so_new     : si + softplus(w)*sign(so)*(main+corr)

Layout "cm" = column-major [128, NC]: n = c*128 + p.
"""

from contextlib import ExitStack

import numpy as np

import concourse.bass as bass
import concourse.mybir as mybir
import concourse.tile as tile
from concourse import bacc, bass_utils
F32 = mybir.dt.float32
F16 = mybir.dt.float16
BF16 = mybir.dt.bfloat16
F8 = mybir.dt.float8e4
I32 = mybir.dt.int32
OP = mybir.AluOpType

NEG_BACK = -60000.0  # "minus infinity" for non-edges (fp16-safe, dominates |so|)

B, M, N, IT = 16, 1024, 2048, 5
N_CORES = 8
BL = B // N_CORES


def build(nc: bass.Bass, M=M, N=N, BL=BL, IT=IT, oh_bufs=10, gp_tt=7, oh_act=8, oh_gp=0, nv_bufs=4, ohs_bufs=2, st_bufs=2, p2_dve=0, halves=1, skip=()):
    MC = M // 128  # m-chunks
    NC = N // 128  # n-chunks

    d_si = nc.dram_tensor("si_cm", [128, NC * BL], F32, kind="ExternalInput").ap()
    d_h = nc.dram_tensor("h_bf", [M, N], BF16, kind="ExternalInput").ap()
    d_hb = nc.dram_tensor("h_big", [M, N], F16, kind="ExternalInput").ap()
    d_ht = nc.dram_tensor("h_t", [N, M], F8, kind="ExternalInput").ap()
    d_misc = nc.dram_tensor("misc_in", [128, 129], F32, kind="ExternalInput").ap()
    d_out = nc.dram_tensor("out", [BL, N], F32, kind="ExternalOutput").ap()

    with tile.TileContext(nc) as tc, ExitStack() as ctx:
        const = ctx.enter_context(tc.tile_pool(name="const", bufs=1))
        state_p = ctx.enter_context(tc.tile_pool(name="state", bufs=st_bufs))
        negv_p = ctx.enter_context(tc.tile_pool(name="negv", bufs=nv_bufs))
        oh_p = ctx.enter_context(tc.tile_pool(name="oh", bufs=oh_bufs))
        ohs_p = ctx.enter_context(tc.tile_pool(name="ohs", bufs=ohs_bufs))
        psum_p = ctx.enter_context(tc.tile_pool(name="ps", bufs=1, space="PSUM"))
        pstr_p = ctx.enter_context(tc.tile_pool(name="pstr", bufs=2, space="PSUM"))
        dram_p = ctx.enter_context(tc.tile_pool(name="dram", bufs=1, space="DRAM"))

        # ---- persistent loads ----
        t_h = const.tile([128, MC * N], BF16)  # H, m-chunk mc at cols [mc*N,(mc+1)*N)
        t_hb = const.tile([128, MC * N], F16)  # (1-H)*NEG_BACK
        t_ht = const.tile([128, NC * M], F8)  # H^T, n-chunk c at cols [c*M,(c+1)*M)
        misc = const.tile([128, 129], F32)  # ident(128) | norm(1)
        ident = misc[:, 0:128]
        t_norm = misc[:, 128:129]
        nc.sync.dma_start(misc[:], d_misc)
        # dummy matmul so the PE observes misc's DMA sem before any transpose
        # (transpose-mode matmuls only support a single sync wait)
        pdum = pstr_p.tile([1, 1], F32, tag="dum", name="dum", bufs=1)
        nc.tensor.matmul(pdum[:], lhsT=ident[:, 0:1], rhs=ident[:, 0:1], start=True, stop=True)
        hb_r = d_hb.rearrange("(mc p) n -> mc p n", p=128)
        h_r = d_h.rearrange("(mc p) n -> mc p n", p=128)
        ht_r = d_ht.rearrange("(c p) m -> c p m", p=128)
        for mc in range(MC):
            nc.sync.dma_start(t_hb[:, mc * N : (mc + 1) * N], hb_r[mc])
        for c in range(NC):
            nc.sync.dma_start(t_ht[:, c * M : (c + 1) * M], ht_r[c])
        for mc in range(MC):
            nc.sync.dma_start(t_h[:, mc * N : (mc + 1) * N], h_r[mc])

        t_si = const.tile([128, NC * BL], F32)
        nc.sync.dma_start(t_si[:], d_si)
        si = [t_si[:, NC * b : NC * (b + 1)] for b in range(BL)]

        abc_p = ctx.enter_context(tc.tile_pool(name="abcp", bufs=2))
        abc = [None] * BL
        at_sb = [const.tile([NC, 128], F32, tag=f"atsb{b}", name=f"atsb{b}") for b in range(BL)]
        d_arow = dram_p.tile([BL, N], F32, name="d_arow")

        def derive_state(so_ap, b, negs):
            """From so (cm [128, NC]) write A, S (f32) and neg (fp8, col 2c+b)."""
            st = state_p.tile([128, 2 * NC], F32, tag=f"st{b}", name=f"st{b}")
            A = st[:, 0:NC]
            S = st[:, NC : 2 * NC]
            nc.vector.tensor_scalar(
                out=A.bitcast(I32), in0=so_ap.bitcast(I32),
                scalar1=0x7FFFFFFF, scalar2=None, op0=OP.bitwise_and,
            )
            nc.vector.tensor_scalar(out=S, in0=so_ap, scalar1=0.0, scalar2=2.0, op0=OP.is_ge, op1=OP.mult)
            nc.vector.tensor_scalar(out=S, in0=S, scalar1=-1.0, scalar2=None, op0=OP.add)
            nc.vector.tensor_scalar(
                out=negs[:].rearrange("p (c two) -> p c two", two=2)[:, :, b : b + 1],
                in0=so_ap.unsqueeze(2),
                scalar1=0.0, scalar2=None, op0=OP.is_lt,
            )
            return A, S

        def bcast_A(A, b):
            """A (cm [128, NC]) -> abc[b] [128, N] row-major broadcast."""
            pt = pstr_p.tile([NC, 128], F32, tag="tr", name="tr")
            nc.tensor.transpose(pt[:], A, ident)
            nc.vector.tensor_copy(at_sb[b][:], pt[:])
            nc.sync.dma_start(d_arow[b : b + 1, :], at_sb[b][:])
            abc[b] = abc_p.tile([128, N], F32, tag=f"abc{b}", name=f"abc{b}")
            # split across DMA queues: one 1MB broadcast would serialize on a
            # single queue's bandwidth
            for q in range(4):
                nc.sync.dma_start(
                    abc[b][:, q * (N // 4) : (q + 1) * (N // 4)],
                    d_arow[b : b + 1, q * (N // 4) : (q + 1) * (N // 4)].to_broadcast([128, N // 4]),
                )

        # ---- init state from si ----
        so = [si[b] for b in range(BL)]
        negs = state_p.tile([128, 2 * NC], F8, tag="negs", name="negs")
        AS = [derive_state(so[b], b, negs) for b in range(BL)]
        for b in range(BL):
            bcast_A(AS[b][0], b)

        for it in range(IT):
            # parity: P[m-part, 2*mc+b] = sum_n H^T * neg   (PE, early)
            pp = psum_p.tile([128, 2 * MC], F32, tag="pp", name="pp")
            for mc in range(MC if "parity" not in skip else 0):
                for c in range(NC):
                    nc.tensor.matmul(
                        pp[:, 2 * mc : 2 * mc + 2],
                        lhsT=t_ht[:, c * M + 128 * mc : c * M + 128 * (mc + 1)],
                        rhs=negs[:, 2 * c : 2 * c + 2],
                        start=(c == 0),
                        stop=(c == NC - 1),
                    )
            # smalls: rs | t1 | t2 | tmp | tmp2 (f32); t1/t2 h,m,l (bf16)
            sm = state_p.tile([128, 10 * MC], F32, tag="sm", name="sm")
            rs, t1s, t2s = sm[:, : 2 * MC], sm[:, 2 * MC : 4 * MC], sm[:, 4 * MC : 6 * MC]
            tmp = sm[:, 6 * MC : 8 * MC]
            tmp2 = sm[:, 8 * MC : 10 * MC]
            smb = state_p.tile([128, 12 * MC], BF16, tag="smb", name="smb")
            t1g = [smb[:, 2 * g * MC : 2 * (g + 1) * MC] for g in range(3)]
            t2g = [smb[:, 2 * (g + 3) * MC : 2 * (g + 4) * MC] for g in range(3)]
            ri = state_p.tile([128, 2 * MC], I32, tag="ri", name="ri")
            nc.vector.tensor_copy(ri[:], pp[:])  # exact: P is integer-valued
            nc.vector.tensor_scalar(out=ri[:], in0=ri[:], scalar1=1, scalar2=None, op0=OP.bitwise_and)
            nc.vector.tensor_copy(rs, ri[:])
            nc.vector.tensor_scalar(out=rs, in0=rs, scalar1=-2.0, scalar2=1.0, op0=OP.mult, op1=OP.add)

            vmax = [state_p.tile([128, 8 * MC], F32, tag=f"vm{b}", name=f"vm{b}") for b in range(BL)]
            smb_g = smb[:].rearrange("p (g j) -> p j g", g=6)
            new_negs = state_p.tile([128, 2 * NC], F8, tag="negs", name="negs") if it < IT - 1 else None
            new_so, new_AS = [], []
            for b in range(BL):
                ohs = {}
                for mc in range(MC):
                    nv = negv_p.tile([128, N], F32, tag="nv", name="nv")
                    tt_eng = nc.gpsimd if mc < gp_tt else nc.vector
                    tt_eng.tensor_tensor(
                        out=nv[:], in0=t_hb[:, mc * N : (mc + 1) * N], in1=abc[b][:], op=OP.subtract
                    )
                    if "max8" not in skip:
                        nc.vector.max(out=vmax[b][:, 8 * mc : 8 * mc + 8], in_=nv[:])
                    oh = oh_p.tile([128, N], BF16, tag="oh", name="oh")
                    ohs[mc] = oh
                    if "oh" not in skip:
                        # onehot on the scalar engine:
                        # s = Sign(vmax0 - nv) in {0 (argmin), 1}; oh = 1 - s
                        osg = ohs_p.tile([128, N], BF16, tag="osg", name="osg")
                        nc.scalar.activation(
                            osg[:], nv[:], mybir.ActivationFunctionType.Sign,
                            bias=vmax[b][:, 8 * mc : 8 * mc + 1], scale=-1.0,
                        )
                        if mc < p2_dve:
                            nc.vector.tensor_scalar(
                                out=oh[:], in0=osg[:], scalar1=-1.0, scalar2=1.0,
                                op0=OP.mult, op1=OP.add,
                            )
                        else:
                            nc.scalar.activation(
                                oh[:], osg[:], mybir.ActivationFunctionType.Copy,
                                bias=1.0, scale=-1.0,
                            )
                # t1/t2 + 3-way bf16 splits, in mc-halves so the PE
                # accumulation matmuls can start before the last max8
                vm8 = vmax[b][:].rearrange("p (c k) -> p c k", k=8)
                rs_bv = rs[:].rearrange("p (c two) -> p c two", two=2)
                for h in range(halves):
                    lo, hi = (MC * h) // halves, (MC * (h + 1)) // halves
                    t2s_bv = t2s.rearrange("p (c two) -> p c two", two=2)[:, lo:hi, b : b + 1]
                    nc.vector.tensor_tensor(
                        out=t2s_bv, in0=vm8[:, lo:hi, 0:1], in1=vm8[:, lo:hi, 1:2], op=OP.subtract
                    )
                    nc.vector.tensor_tensor(
                        out=t2s_bv, in0=t2s_bv, in1=rs_bv[:, lo:hi, b : b + 1], op=OP.mult
                    )
                    nc.vector.scalar_tensor_tensor(
                        out=t1s[:].rearrange("p (c two) -> p c two", two=2)[:, lo:hi, b : b + 1],
                        in0=vm8[:, lo:hi, 0:1],
                        scalar=-1.0,
                        in1=rs_bv[:, lo:hi, b : b + 1],
                        op0=OP.mult, op1=OP.mult,
                    )
                    for ts_, tmp_, gs in ((t1s, tmp, t1g), (t2s, tmp2, t2g)):
                        ts_b = ts_.rearrange("p (c two) -> p c two", two=2)[:, lo:hi, b : b + 1]
                        tm_b = tmp_.rearrange("p (c two) -> p c two", two=2)[:, lo:hi, b : b + 1]
                        g_b = [g.rearrange("p (c two) -> p c two", two=2)[:, lo:hi, b : b + 1] for g in gs]
                        nc.vector.tensor_copy(g_b[0], ts_b)
                        nc.vector.tensor_tensor(out=tm_b, in0=ts_b, in1=g_b[0], op=OP.subtract)
                        nc.vector.tensor_copy(g_b[1], tm_b)
                        nc.vector.tensor_tensor(out=g_b[2], in0=tm_b, in1=g_b[1], op=OP.subtract)

                # main(b): pm_b[n-part, 3c+k] = sum_m H*t1{h,m,l}[b]
                pm = psum_p.tile([128, 3 * NC], F32, tag=f"pm{b}", name=f"pm{b}")
                for c in range(NC if "main" not in skip else 0):
                    for mc in range(MC):
                        nc.tensor.matmul(
                            pm[:, 3 * c : 3 * c + 3],
                            lhsT=t_h[:, mc * N + 128 * c : mc * N + 128 * (c + 1)],
                            rhs=smb_g[:, 2 * mc + b : 2 * mc + b + 1, 0:3],
                            start=(mc == 0),
                            stop=(mc == MC - 1),
                        )
                # corr(b): pcr_b[n-part, 3c+k] = sum_m onehot*t2{h,m,l}
                pcr = psum_p.tile([128, 3 * NC], F32, tag=f"pcr{b}", name=f"pcr{b}")
                for c in range(NC if "corr" not in skip else 0):
                    for mc in range(MC):
                        nc.tensor.matmul(
                            pcr[:, 3 * c : 3 * c + 3],
                            lhsT=ohs[mc][:, 128 * c : 128 * (c + 1)],
                            rhs=smb_g[:, 2 * mc + b : 2 * mc + b + 1, 3:6],
                            start=(mc == 0),
                            stop=(mc == MC - 1),
                        )

                # combine(b)
                A_old, S_old = AS[b]
                st2 = state_p.tile([128, 3 * NC], F32, tag=f"cmb{b}", name=f"cmb{b}")
                crr, c1, so_n = st2[:, :NC], st2[:, NC : 2 * NC], st2[:, 2 * NC :]
                pcr_v = pcr[:].rearrange("p (c three) -> p c three", three=3)
                pm_v = pm[:].rearrange("p (c three) -> p c three", three=3)
                # accumulate one PSUM operand at a time (ISA: <=1 PSUM input/op)
                nc.vector.tensor_copy(crr.unsqueeze(2), pcr_v[:, :, 0:1])
                for k in (1, 2):
                    nc.vector.tensor_tensor(
                        out=crr.unsqueeze(2), in0=crr.unsqueeze(2),
                        in1=pcr_v[:, :, k : k + 1], op=OP.add,
                    )
                nc.vector.tensor_tensor(
                    out=c1.unsqueeze(2), in0=crr.unsqueeze(2), in1=pm_v[:, :, 0:1], op=OP.add
                )
                for k in (1, 2):
                    nc.vector.tensor_tensor(
                        out=c1.unsqueeze(2), in0=c1.unsqueeze(2), in1=pm_v[:, :, k : k + 1], op=OP.add
                    )
                nc.vector.scalar_tensor_tensor(
                    out=c1, in0=c1, scalar=t_norm, in1=S_old, op0=OP.mult, op1=OP.mult
                )
                nc.vector.tensor_tensor(out=so_n, in0=si[b], in1=c1, op=OP.add)
                new_so.append(so_n)
                if it < IT - 1:
                    A_n, S_n = derive_state(so_n, b, new_negs)
                    new_AS.append((A_n, S_n))
                    bcast_A(A_n, b)
            so = new_so
            if it < IT - 1:
                AS = new_AS
                negs = new_negs

        # output: so (cm) -> row-major [BL, N]
        for b in range(BL):
            po = pstr_p.tile([NC, 128], F32, tag="tr", name="tr")
            nc.tensor.transpose(po[:], so[b], ident)
            nc.vector.tensor_copy(at_sb[b][:], po[:])
            nc.sync.dma_start(d_out[b : b + 1, :], at_sb[b][:])

    return nc


_CACHE = {}


def _get_nc():
    if "nc" not in _CACHE:
        nc = bacc.Bacc("TRN2", target_bir_lowering=False)
        build(nc)
        nc.compile()
        _CACHE["nc"] = nc
    return _CACHE["nc"]


def _cm(row, ncnk):  # [N] -> [128, ncnk] column-major
    return row.reshape(ncnk, 128).T


def kernel(soft_input, H, labels, w):
    del labels  # unused by the reference computation
    soft_input = np.asarray(soft_input, dtype=np.float32)
    H = np.asarray(H)
    w = np.asarray(w, dtype=np.float32)
    NC = N // 128

    norm = np.log1p(np.exp(np.float64(w[0]))).astype(np.float32)
    f8 = mybir.dt.np(F8)
    bf = mybir.dt.np(BF16)
    h_bf = H.astype(bf)
    h_big = ((1 - H) * NEG_BACK).astype(np.float16)
    h_t = np.ascontiguousarray(H.T).astype(f8)
    misc_in = np.concatenate(
        [np.eye(128, dtype=np.float32), np.full((128, 1), norm, dtype=np.float32)], axis=1
    )

    in_maps = []
    for core in range(N_CORES):
        rows = soft_input[BL * core : BL * (core + 1)]
        si_cm = np.concatenate([_cm(rows[b], NC) for b in range(BL)], axis=1)
        in_maps.append(
            {
                "si_cm": np.ascontiguousarray(si_cm, dtype=np.float32),
                "h_bf": h_bf,
                "h_big": h_big,
                "h_t": h_t,
                "misc_in": misc_in,
            }
        )

    nc = _get_nc()
    res = bass_utils.run_bass_kernel_spmd(nc, in_maps, core_ids=list(range(N_CORES)))
    out = np.concatenate([r["out"] for r in res.results], axis=0)
    return out.astype(np.float32)
